# revision 1
# baseline (speedup 1.0000x reference)
"""DreamFit single-stream processor block on 8 Trainium2 NeuronCores.

Sharding: 3 heads per core for SDPA (column-parallel linear1), 1/8 of the MLP
per core, row-parallel linear2 (host sums the 8 partial outputs). LoRA branches
are folded into w1/w2 on the host (lora_weight == 1). The modulation matvec is
sharded 8-way and AllGathered on device. All big matmuls run in float32r
(TF32-like) at full PE rate; fp32r inputs to DVE ops are read through fp32
bitcast views and rounded on write.
"""
import math
import os
from contextlib import ExitStack

import numpy as np

import concourse.bass as bass
import concourse.mybir as mybir
import concourse.tile as tile
from concourse import bacc
from concourse.bass_utils import run_bass_kernel_spmd
from concourse.masks import make_identity

F32 = mybir.dt.float32
F32R = mybir.dt.float32r
AF = mybir.ActivationFunctionType
ALU = mybir.AluOpType

P = 128
HID = 3072
HEADS = 24
HD = 128
MLP = 4 * HID            # 12288
L = 2048
NCORES = 8
H_PER = HEADS // NCORES  # 3 heads per core
DQK = H_PER * HD         # 384 q (and k, v) out dims per core
DMLP = MLP // NCORES     # 1536 mlp dims per core
DOUT1 = 3 * DQK + DMLP   # 2688 linear1 out dims per core
NBLK1 = DOUT1 // P       # 21 (0-2 q, 3-5 k, 6-8 v, 9-20 mlp)
CATD = DQK + DMLP        # 1920 cat dims per core
NCAT = CATD // P         # 15
MODSH = 3 * HID // NCORES  # 1152 modulation outputs per core
NMOD = MODSH // P        # 9
HC = HID // P            # 24 hidden chunks
NQ = 4                   # token quarters
LQ = L // NQ             # 512
LB = LQ // P             # 4 token tiles per quarter
NKB = L // P             # 16 key blocks
EPS = 1e-6

_CACHED = {}


def _build_nc():
    nc = bacc.Bacc("TRN2", target_bir_lowering=False, debug=False,
                   num_devices=NCORES)
    x_in = nc.dram_tensor("x_in", [L, HID], F32R, kind="ExternalInput")
    vec_in = nc.dram_tensor("vec_in", [HID], F32, kind="ExternalInput")
    cs_in = nc.dram_tensor("cs_in", [P, L], F32, kind="ExternalInput")  # cos|sin
    csw_in = nc.dram_tensor("csw_in", [P, L], F32, kind="ExternalInput")  # sin|cos
    w1t_in = nc.dram_tensor("w1t_in", [HID, DOUT1], F32R, kind="ExternalInput")
    b1_in = nc.dram_tensor("b1_in", [DOUT1], F32, kind="ExternalInput")
    w2t_in = nc.dram_tensor("w2t_in", [CATD, HID], F32R, kind="ExternalInput")
    b2_in = nc.dram_tensor("b2_in", [HID], F32, kind="ExternalInput")  # zeros off core0
    mwt_in = nc.dram_tensor("mwt_in", [HID, MODSH], F32R, kind="ExternalInput")
    qs_in = nc.dram_tensor("qs_in", [HD], F32, kind="ExternalInput")  # permuted, /sqrt(HD)
    ks_in = nc.dram_tensor("ks_in", [HD], F32, kind="ExternalInput")  # permuted
    out_t = nc.dram_tensor("out_part", [HC, P, L], F32, kind="ExternalOutput")

    with tile.TileContext(nc) as tc, \
            nc.allow_low_precision(reason="fp32r (tf32) matmul pipeline is intentional"):
        _emit(nc, tc, x_in, vec_in, cs_in, csw_in, w1t_in, b1_in, w2t_in, b2_in,
              mwt_in, qs_in, ks_in, out_t)
    nc.compile()
    return nc


def _emit(nc, tc, x_in, vec_in, cs_in, csw_in, w1t_in, b1_in, w2t_in, b2_in,
          mwt_in, qs_in, ks_in, out_t):
    _PH = os.environ.get("KPHASES", "ABCEF")
    with ExitStack() as top:
        const = top.enter_context(tc.tile_pool(name="const", bufs=1))
        dram = top.enter_context(tc.tile_pool(name="dram", bufs=1, space="DRAM"))
        modp = top.enter_context(tc.tile_pool(name="modp", bufs=1))
        psum = top.enter_context(tc.tile_pool(name="psum", bufs=4, space="PSUM"))
        pscol = top.enter_context(tc.tile_pool(name="pscol", bufs=2, space="PSUM"))
        pstr = top.enter_context(tc.tile_pool(name="pstr", bufs=2, space="PSUM"))

        # ---- constants ----
        ident_f = const.tile([P, P], F32)
        make_identity(nc, ident_f)
        ident = const.tile([P, P], F32R)
        nc.vector.tensor_copy(ident, ident_f)
        ones_c_f = const.tile([P, 1], F32)
        nc.vector.memset(ones_c_f, 1.0)
        ones_c = const.tile([P, 1], F32R)         # K=128, M=1
        nc.vector.tensor_copy(ones_c, ones_c_f)
        ones_r_f = const.tile([1, P], F32)
        nc.vector.memset(ones_r_f, 1.0)
        ones_r = const.tile([1, P], F32R)         # K=1, M=128
        nc.vector.tensor_copy(ones_r, ones_r_f)
        eps_c = const.tile([P, 1], F32)
        nc.vector.memset(eps_c, EPS)
        eps_1 = const.tile([1, 1], F32)
        nc.vector.memset(eps_1, EPS)
        cs = const.tile([P, L], F32)              # rows 0-63 cos, 64-127 sin
        nc.sync.dma_start(out=cs, in_=cs_in[:, :])
        csw = const.tile([P, L], F32)             # rows 0-63 sin, 64-127 cos
        nc.sync.dma_start(out=csw, in_=csw_in[:, :])
        qs = const.tile([P, 1], F32)
        nc.sync.dma_start(out=qs, in_=qs_in[:, None])
        ks = const.tile([P, 1], F32)
        nc.sync.dma_start(out=ks, in_=ks_in[:, None])
        b1t = const.tile([P, NBLK1], F32)
        nc.sync.dma_start(out=b1t, in_=b1_in.rearrange("(b p) -> p b", p=P))
        b2t = const.tile([P, HC], F32)
        nc.sync.dma_start(out=b2t, in_=b2_in.rearrange("(b p) -> p b", p=P))

        # persistent small modulation tiles
        scale1p = modp.tile([P, HC], F32)
        shift_s = modp.tile([P, HC], F32)
        gate_t = modp.tile([P, HC], F32)
        btot = modp.tile([P, NBLK1], F32)
        gelT_d = dram.tile([NBLK1 - 9, P, L], F32R)
        atp = top.enter_context(tc.tile_pool(name="attn", bufs=1))

        with ExitStack() as bc_scope:
            qkv = bc_scope.enter_context(tc.tile_pool(name="qkv", bufs=1))
            qkT = [qkv.tile([P, L], F32R, tag=f"q{h}", name=f"q{h}") for h in range(H_PER)] + \
                  [qkv.tile([P, L], F32R, tag=f"k{h}", name=f"k{h}") for h in range(H_PER)]
            vblk = [[None] * NKB for _ in range(H_PER)]

            # ============================================================
            # Phase A: modulation matvec (sharded) + AllGather
            # ============================================================
            with ExitStack() as ab:
              if "A" in _PH:
                w1p = ab.enter_context(tc.tile_pool(name="w1s", bufs=2))
                svf = modp.tile([P, HC], F32)
                nc.sync.dma_start(out=svf, in_=vec_in.rearrange("(c p) -> p c", p=P))
                sv = modp.tile([P, HC], F32)
                nc.scalar.activation(sv, svf, AF.Silu)
                mloc = modp.tile([P, NMOD], F32)
                mwt_r = mwt_in.rearrange("(c p) m -> p c m", p=P)
                for blk in range(NMOD):
                    mwt = w1p.tile([P, HC, P], F32R, tag="w1t")
                    nc.sync.dma_start(out=mwt, in_=mwt_r[:, :, blk * P:(blk + 1) * P])
                    ps = pscol.tile([P, 1], F32, tag="col")
                    for hc in range(HC):
                        nc.tensor.matmul(ps, mwt[:, hc].bitcast(F32),
                                         sv[:, hc:hc + 1],
                                         start=(hc == 0), stop=(hc == HC - 1))
                    nc.scalar.copy(mloc[:, blk:blk + 1], ps)
                m_shard = dram.tile([MODSH], F32)
                nc.sync.dma_start(out=m_shard.rearrange("(b p) -> p b", p=P), in_=mloc)
                m_full = dram.tile([3 * HID], F32)
                if os.environ.get("KNOCOLL"):
                    nc.sync.dma_start(
                        out=m_full[0:MODSH].rearrange("(b p) -> p b", p=P), in_=mloc)
                else:
                    nc.gpsimd.collective_compute(
                        "AllGather", ALU.bypass, replica_groups=[list(range(NCORES))],
                        ins=[m_shard.opt()], outs=[m_full.opt()])
                nc.sync.dma_start(out=scale1p,
                                  in_=m_full[HID:2 * HID].rearrange("(c p) -> p c", p=P))
                nc.vector.tensor_scalar_add(scale1p, scale1p, 1.0)
                nc.sync.dma_start(out=shift_s,
                                  in_=m_full[0:HID].rearrange("(c p) -> p c", p=P))
                nc.sync.dma_start(out=gate_t,
                                  in_=m_full[2 * HID:3 * HID].rearrange("(c p) -> p c", p=P))

                # ============================================================
                # Phase B: per quarter: LN -> x_modT -> linear1 -> (v transp)
                # ============================================================
                if "B" not in _PH:
                    return
                lnp = ab.enter_context(tc.tile_pool(name="ln", bufs=1))
                xmp = ab.enter_context(tc.tile_pool(name="xm", bufs=1))
                gsp = ab.enter_context(tc.tile_pool(name="gsp", bufs=2))
                vqp = ab.enter_context(tc.tile_pool(name="vq", bufs=2))
                x_r = x_in.rearrange("(t p) h -> t p h", p=P)
                w1t_r = w1t_in.rearrange("(c p) m -> p c m", p=P)

                for q in range(NQ):
                    qsl = slice(q * LQ, (q + 1) * LQ)
                    xmT = xmp.tile([P, HC, LQ], F32R, tag="xmT")
                    for lb in range(LB):
                        t = q * LB + lb
                        xt = lnp.tile([P, HID], F32R, tag="xt")
                        nc.sync.dma_start(out=xt, in_=x_r[t])
                        xtf = xt.bitcast(F32)
                        stats = lnp.tile([P, 6, 6], F32, tag="stats")
                        for sg in range(6):
                            nc.vector.bn_stats(out=stats[:, sg, :],
                                               in_=xtf[:, sg * 512:(sg + 1) * 512])
                        mv = lnp.tile([P, 2], F32, tag="mv")
                        nc.vector.bn_aggr(out=mv, in_=stats)
                        std = lnp.tile([P, 1], F32, tag="std")
                        nc.scalar.activation(std, mv[:, 1:2], AF.Sqrt, bias=eps_c)
                        rstd = lnp.tile([P, 1], F32, tag="rstd")
                        nc.vector.reciprocal(rstd, std)
                        # in-place: xt <- (xt - mean) * rstd, rounded to fp32r
                        nc.vector.tensor_scalar(xt, xtf, mv[:, 0:1],
                                                rstd, ALU.subtract, ALU.mult)
                        for hcc in range(HC):
                            pt = pstr.tile([P, P], F32, tag="tr")
                            nc.tensor.transpose(
                                pt.bitcast(F32R),
                                xt[:, hcc * P:(hcc + 1) * P], ident)
                            nc.scalar.activation(
                                xmT[:, hcc, lb * P:(lb + 1) * P], pt, AF.Copy,
                                scale=scale1p[:, hcc:hcc + 1])
                    # ---- linear1 on this quarter ----
                    for blk in range(NBLK1):
                        w1t = w1p.tile([P, HC, P], F32R, tag="w1t")
                        nc.sync.dma_start(out=w1t,
                                          in_=w1t_r[:, :, blk * P:(blk + 1) * P])
                        ps = psum.tile([P, LQ], F32, tag="big")
                        for hc in range(HC):
                            nc.tensor.matmul(ps, w1t[:, hc], xmT[:, hc, :],
                                             start=(hc == 0), stop=(hc == HC - 1))
                        if q == 0:
                            psb = pscol.tile([P, 1], F32, tag="col")
                            for hc in range(HC):
                                nc.tensor.matmul(psb, w1t[:, hc].bitcast(F32),
                                                 shift_s[:, hc:hc + 1],
                                                 start=(hc == 0), stop=(hc == HC - 1))
                            nc.vector.tensor_tensor(btot[:, blk:blk + 1], psb,
                                                    b1t[:, blk:blk + 1], ALU.add)
                        if blk < 6:       # q / k
                            nc.vector.tensor_scalar_add(qkT[blk][:, qsl], ps,
                                                        btot[:, blk:blk + 1])
                        elif blk < 9:     # v: evict then transpose to [l, d]
                            h = blk - 6
                            vq = vqp.tile([P, LQ], F32R, tag="vq")
                            nc.vector.tensor_scalar_add(vq, ps, btot[:, blk:blk + 1])
                            for j in range(LB):
                                ptv = pstr.tile([P, P], F32, tag="tr")
                                nc.tensor.transpose(ptv.bitcast(F32R),
                                                    vq[:, j * P:(j + 1) * P], ident)
                                vb = qkv.tile([P, P], F32R, tag=f"vb{h}_{q * LB + j}", name=f"vb{h}_{q * LB + j}")
                                nc.scalar.copy(vb, ptv)
                                vblk[h][q * LB + j] = vb
                        else:             # mlp -> gelu -> DRAM spill
                            g = gsp.tile([P, LQ], F32R, tag="gel")
                            nc.scalar.activation(g, ps, AF.Gelu_apprx_tanh,
                                                 bias=btot[:, blk:blk + 1])
                            nc.sync.dma_start(out=gelT_d[blk - 9, :, qsl], in_=g)

            # ============================================================
            # Phase C: QK-norm (RMS over head dim) + rope
            # ============================================================
            with ExitStack() as cc:
              if "C" in _PH:
                rmsp = cc.enter_context(tc.tile_pool(name="rms", bufs=2))
                srp = cc.enter_context(tc.tile_pool(name="srp", bufs=1))
                rtp = cc.enter_context(tc.tile_pool(name="rtp", bufs=4))
                for i, t in enumerate(qkT):
                    scale_ap = qs if i < H_PER else ks
                    tf = t.bitcast(F32)
                    sq = rmsp.tile([P, L], F32R, tag="sq")
                    nc.vector.tensor_mul(sq, tf, tf)
                    srt = srp.tile([1, L], F32, tag="srt")
                    for j in range(NQ):
                        pscol_t = pscol.tile([1, LQ], F32, tag="col")
                        nc.tensor.matmul(pscol_t, ones_c, sq[:, j * LQ:(j + 1) * LQ],
                                         start=True, stop=True)
                        nc.scalar.activation(srt[:, j * LQ:(j + 1) * LQ], pscol_t,
                                             AF.Sqrt, bias=eps_1, scale=1.0 / HD)
                    rinv = srp.tile([1, L], F32R, tag="rinv")
                    nc.vector.reciprocal(rinv, srt)
                    rb = rmsp.tile([P, L], F32, tag="rb")
                    for j in range(NQ):
                        pb = psum.tile([P, LQ], F32, tag="big")
                        nc.tensor.matmul(pb, ones_r, rinv[:, j * LQ:(j + 1) * LQ],
                                         start=True, stop=True)
                        nc.scalar.copy(rb[:, j * LQ:(j + 1) * LQ], pb)
                    nc.vector.tensor_mul(t, tf, rb)
                    nc.vector.tensor_scalar_mul(t, tf, scale_ap)
                    # rope: rows 0-63 even pair components, 64-127 odd.
                    # All DVE operands must share a base partition, so stage
                    # products in full-height tiles and realign the crossing
                    # halves with an SBUF->SBUF DMA partition shift.
                    te, to = t[0:64, :], t[64:128, :]
                    tef, tof = tf[0:64, :], tf[64:128, :]
                    A = rtp.tile([P, L], F32, tag="rt")   # [qe*cos ; qo*cos]
                    B = rtp.tile([P, L], F32, tag="rt")   # [qe*sin ; qo*sin]
                    Bx = rtp.tile([P, L], F32, tag="rt")  # [qo*sin ; qe*sin]
                    nc.vector.tensor_mul(A[0:64, :], tef, cs[0:64, :])
                    nc.vector.tensor_mul(A[64:128, :], tof, csw[64:128, :])
                    nc.vector.tensor_mul(B[0:64, :], tef, csw[0:64, :])
                    nc.vector.tensor_mul(B[64:128, :], tof, cs[64:128, :])
                    nc.sync.dma_start(out=Bx[0:64, :], in_=B[64:128, :])
                    nc.sync.dma_start(out=Bx[64:128, :], in_=B[0:64, :])
                    nc.vector.tensor_tensor(te, A[0:64, :], Bx[0:64, :], ALU.subtract)
                    nc.vector.tensor_tensor(to, Bx[64:128, :], A[64:128, :], ALU.add)

            # ============================================================
            # Phase E: attention per head (scoresT -> exp -> denom -> outT)
            # ============================================================
            aoT = [atp.tile([P, L], F32R, tag=f"ao{h}", name=f"ao{h}") for h in range(H_PER)]
            with ExitStack() as ec:
              if "E" in _PH:
                ptp = ec.enter_context(tc.tile_pool(name="ptp", bufs=20))
                sdp = ec.enter_context(tc.tile_pool(name="sdp", bufs=2))
                for h in range(H_PER):
                    qT, kT = qkT[h], qkT[H_PER + h]
                    for qc in range(NQ):
                        qsl = slice(qc * LQ, (qc + 1) * LQ)
                        pts = []
                        for kb in range(NKB):
                            ps = psum.tile([P, LQ], F32, tag="big")
                            nc.tensor.matmul(ps, kT[:, kb * P:(kb + 1) * P],
                                             qT[:, qsl], start=True, stop=True)
                            ptile = ptp.tile([P, LQ], F32R, tag="pt", name="pt")
                            nc.scalar.activation(ptile, ps, AF.Exp)
                            pts.append(ptile)
                        psd = pscol.tile([1, LQ], F32, tag="col")
                        for kb in range(NKB):
                            nc.tensor.matmul(psd, ones_c, pts[kb],
                                             start=(kb == 0), stop=(kb == NKB - 1))
                        rd = sdp.tile([1, LQ], F32R, tag="rd")
                        nc.vector.reciprocal(rd, psd)
                        pbd = psum.tile([P, LQ], F32, tag="big")
                        nc.tensor.matmul(pbd, ones_r, rd, start=True, stop=True)
                        rbd = sdp.tile([P, LQ], F32, tag="rbd")
                        nc.scalar.copy(rbd, pbd)
                        pso = psum.tile([P, LQ], F32, tag="big")
                        for kb in range(NKB):
                            nc.tensor.matmul(pso, vblk[h][kb], pts[kb],
                                             start=(kb == 0), stop=(kb == NKB - 1))
                        nc.vector.tensor_mul(aoT[h][:, qsl], pso, rbd)

        # ============================================================
        # Phase F: linear2 (row-parallel partial) with gate; +b2 on core 0
        # ============================================================
        with ExitStack() as fc:
          if "F" in _PH:
            glp = fc.enter_context(tc.tile_pool(name="glp", bufs=1))
            w2p = fc.enter_context(tc.tile_pool(name="w2p", bufs=2))
            otp = fc.enter_context(tc.tile_pool(name="otp", bufs=3))
            gelT = [glp.tile([P, L], F32R, tag=f"gl{i}", name=f"gl{i}") for i in range(NBLK1 - 9)]
            for i in range(NBLK1 - 9):
                nc.sync.dma_start(out=gelT[i], in_=gelT_d[i])
            catT = aoT + gelT  # 15 chunks of [128, L]
            w2t_r = w2t_in.rearrange("(c p) m -> p c m", p=P)
            for blk in range(HC):
                w2t = w2p.tile([P, NCAT, P], F32R, tag="w2t")
                nc.sync.dma_start(out=w2t, in_=w2t_r[:, :, blk * P:(blk + 1) * P])
                for lc in range(NQ):
                    ps = psum.tile([P, LQ], F32, tag="big")
                    for hc in range(NCAT):
                        nc.tensor.matmul(ps, w2t[:, hc],
                                         catT[hc][:, lc * LQ:(lc + 1) * LQ],
                                         start=(hc == 0), stop=(hc == NCAT - 1))
                    ot = otp.tile([P, LQ], F32, tag="ot")
                    nc.vector.tensor_scalar(ot, ps, b2t[:, blk:blk + 1],
                                            gate_t[:, blk:blk + 1], ALU.add, ALU.mult)
                    nc.sync.dma_start(out=out_t[blk, :, lc * LQ:(lc + 1) * LQ], in_=ot)


def _host_prep(inputs):
    perm = np.concatenate([np.arange(0, HD, 2), np.arange(1, HD, 2)])
    w1 = inputs["w1"].astype(np.float32)
    w1_eff = w1.copy()
    for i, nm in enumerate(("q", "k", "v")):
        up = inputs[f"lora_{nm}_up"].astype(np.float32)
        dn = inputs[f"lora_{nm}_down"].astype(np.float32)
        w1_eff[i * HID:(i + 1) * HID] += up @ dn
    b1 = inputs["b1"].astype(np.float32)
    w2 = inputs["w2"].astype(np.float32)
    w2_eff = w2 + inputs["proj_up"].astype(np.float32) @ \
        inputs["proj_down"].astype(np.float32)
    mod_w = inputs["mod_w"].astype(np.float32)
    mod_b = inputs["mod_b"].astype(np.float32)
    if np.abs(mod_b).max() != 0.0:
        raise NotImplementedError("nonzero mod_b not supported")

    wq = w1_eff[0:HID].reshape(HEADS, HD, HID)[:, perm, :]
    wk = w1_eff[HID:2 * HID].reshape(HEADS, HD, HID)[:, perm, :]
    wv = w1_eff[2 * HID:3 * HID].reshape(HEADS, HD, HID)
    wm = w1_eff[3 * HID:].reshape(NCORES, DMLP, HID)
    bq = b1[0:HID].reshape(HEADS, HD)[:, perm]
    bk = b1[HID:2 * HID].reshape(HEADS, HD)[:, perm]
    bv = b1[2 * HID:3 * HID].reshape(HEADS, HD)
    bm = b1[3 * HID:].reshape(NCORES, DMLP)

    pe = inputs["pe"].astype(np.float32)
    cos = pe[0, 0, :, :, 0, 0]   # (L, 64)
    sin = pe[0, 0, :, :, 1, 0]   # (L, 64)
    cs = np.ascontiguousarray(np.concatenate([cos.T, sin.T], axis=0))  # (128, L)
    csw = np.ascontiguousarray(np.concatenate([sin.T, cos.T], axis=0))  # (128, L)

    qsc = inputs["q_scale"].astype(np.float32)[perm] / math.sqrt(HD)
    ksc = inputs["k_scale"].astype(np.float32)[perm]
    x2d = np.ascontiguousarray(inputs["x"].astype(np.float32).reshape(L, HID))
    vecv = np.ascontiguousarray(inputs["vec"].astype(np.float32).reshape(HID))
    b2 = inputs["b2"].astype(np.float32)

    in_maps = []
    for c in range(NCORES):
        hs = slice(H_PER * c, H_PER * (c + 1))
        w1s = np.concatenate([
            wq[hs].reshape(DQK, HID), wk[hs].reshape(DQK, HID),
            wv[hs].reshape(DQK, HID), wm[c]], axis=0)
        b1s = np.concatenate([
            bq[hs].reshape(DQK), bk[hs].reshape(DQK), bv[hs].reshape(DQK), bm[c]])
        w2s = np.concatenate([
            w2_eff[:, DQK * c:DQK * (c + 1)],
            w2_eff[:, HID + DMLP * c:HID + DMLP * (c + 1)]], axis=1)
        assert w2s.shape == (HID, CATD), w2s.shape
        in_maps.append({
            "x_in": x2d,
            "vec_in": vecv,
            "cs_in": cs,
            "csw_in": csw,
            "w1t_in": np.ascontiguousarray(w1s.T),
            "b1_in": np.ascontiguousarray(b1s),
            "w2t_in": np.ascontiguousarray(w2s.T),
            "b2_in": b2 if c == 0 else np.zeros_like(b2),
            "mwt_in": np.ascontiguousarray(mod_w[MODSH * c:MODSH * (c + 1)].T),
            "qs_in": qsc,
            "ks_in": ksc,
        })
    return in_maps


def kernel(**inputs):
    if "nc" not in _CACHED:
        _CACHED["nc"] = _build_nc()
    nc = _CACHED["nc"]
    in_maps = _host_prep(inputs)
    res = run_bass_kernel_spmd(nc, in_maps, core_ids=list(range(NCORES)))
    acc = np.zeros((HID, L), dtype=np.float64)
    for c in range(NCORES):
        acc += res.results[c]["out_part"].reshape(HID, L)
    out = inputs["x"].astype(np.float32).reshape(L, HID) + acc.T.astype(np.float32)
    return out.reshape(1, L, HID).astype(np.float32)



# revision 2
# speedup vs baseline: 1.0251x; 1.0251x over previous
"""DreamFit single-stream processor block on 8 Trainium2 NeuronCores — v2.

Changes vs v1 (2.27 ms HW):
- bf16 matmul pipeline (weights, activations) instead of fp32r: halves
  weight DMA (w1 stream 66MB vs 132MB) and doubles DVE throughput on the
  rms/rope elementwise chain.
- Double-buffered x_modT quarters so LN/transpose of quarter q+1 overlaps
  linear1 matmuls of quarter q (kills the per-quarter PE stalls that kept
  dropping the HAM clock to 4/8).
- 1/x via scalar-engine exp(-ln(x)) instead of DVE iterative reciprocal
  (the [1,N] reciprocals cost 120us of single-lane DVE time in v1).
- RMS scale (q_scale/sqrt(HD), k_scale) folded into the rinv broadcast
  eviction, saving a DVE pass per qk tile.
- Modulation matvec as 72 wide matmuls (moving=weights) instead of 216
  ap=1 matvecs.
- rms/rope emitted interleaved with the last quarter's MLP blocks so the
  DVE work hides under PE matmuls; attention follows with no LN gap.
"""
import math
import os
from contextlib import ExitStack

import numpy as np
import ml_dtypes

import concourse.bass as bass
import concourse.mybir as mybir
import concourse.tile as tile
from concourse import bacc
from concourse.bass_utils import run_bass_kernel_spmd
from concourse.masks import make_identity

F32 = mybir.dt.float32
BF = mybir.dt.bfloat16
I32 = mybir.dt.int32
AF = mybir.ActivationFunctionType
ALU = mybir.AluOpType

P = 128
HID = 3072
HEADS = 24
HD = 128
MLP = 4 * HID            # 12288
L = 2048
NCORES = 8
H_PER = HEADS // NCORES  # 3 heads per core
DQK = H_PER * HD         # 384 q (and k, v) out dims per core
DMLP = MLP // NCORES     # 1536 mlp dims per core
DOUT1 = 3 * DQK + DMLP   # 2688 linear1 out dims per core
NBLK1 = DOUT1 // P       # 21 (0-2 q, 3-5 k, 6-8 v, 9-20 mlp)
CATD = DQK + DMLP        # 1920 cat dims per core
NCAT = CATD // P         # 15
MODSH = 3 * HID // NCORES  # 1152 modulation outputs per core
HC = HID // P            # 24 hidden chunks
NQ = 4                   # token quarters
LQ = L // NQ             # 512
LB = LQ // P             # 4 token tiles per quarter
NKB = L // P             # 16 key blocks
EPS = 1e-6

_CACHED = {}


def _pin_lnexp_tables():
    """Make the act-table picker put Exp and Ln in the one table that has
    both, so Exp->Ln->Exp chains (rms rsqrt, softmax 1/x) don't thrash
    ACT_TABLE_LOADs (1.3us each, on the attention critical path)."""
    orig = bacc.get_activation_tables

    def patched(arch):
        tabs = {k: set(v) for k, v in orig(arch).items()}
        for name, s in tabs.items():
            if name != "natural_log_exp_and_others":
                s.discard(AF.Exp)
                s.discard(AF.Ln)
        return tabs

    bacc.get_activation_tables = patched
    return orig


def _build_nc():
    _orig_tables = _pin_lnexp_tables()
    nc = bacc.Bacc("TRN2", target_bir_lowering=False, debug=False,
                   num_devices=NCORES)
    x_in = nc.dram_tensor("x_in", [L, HID], F32, kind="ExternalInput")
    vec_in = nc.dram_tensor("vec_in", [HID], F32, kind="ExternalInput")
    cs_in = nc.dram_tensor("cs_in", [P, L], BF, kind="ExternalInput")  # cos|sin
    csw_in = nc.dram_tensor("csw_in", [P, L], BF, kind="ExternalInput")  # sin|cos
    w1t_in = nc.dram_tensor("w1t_in", [HID, DOUT1], BF, kind="ExternalInput")
    b1_in = nc.dram_tensor("b1_in", [DOUT1], F32, kind="ExternalInput")
    w2t_in = nc.dram_tensor("w2t_in", [CATD, HID], BF, kind="ExternalInput")
    b2_in = nc.dram_tensor("b2_in", [HID], F32, kind="ExternalInput")  # zeros off core0
    mwt_in = nc.dram_tensor("mwt_in", [HID, MODSH], BF, kind="ExternalInput")
    qs_in = nc.dram_tensor("qs_in", [HD], F32, kind="ExternalInput")  # permuted, /sqrt(HD)
    ks_in = nc.dram_tensor("ks_in", [HD], F32, kind="ExternalInput")  # permuted
    out_t = nc.dram_tensor("out_part", [HC, P, L], F32, kind="ExternalOutput")

    with tile.TileContext(nc) as tc, \
            nc.allow_low_precision(reason="bf16 matmul pipeline is intentional"):
        _emit(nc, tc, x_in, vec_in, cs_in, csw_in, w1t_in, b1_in, w2t_in, b2_in,
              mwt_in, qs_in, ks_in, out_t)
    nc.compile()
    bacc.get_activation_tables = _orig_tables
    return nc


def _emit(nc, tc, x_in, vec_in, cs_in, csw_in, w1t_in, b1_in, w2t_in, b2_in,
          mwt_in, qs_in, ks_in, out_t):
    with ExitStack() as top:
        const = top.enter_context(tc.tile_pool(name="const", bufs=1))
        dram = top.enter_context(tc.tile_pool(name="dram", bufs=1, space="DRAM"))
        modp = top.enter_context(tc.tile_pool(name="modp", bufs=1))
        psum = top.enter_context(tc.tile_pool(name="psum", bufs=4, space="PSUM"))
        pscol = top.enter_context(tc.tile_pool(name="pscol", bufs=2, space="PSUM"))
        pstr = top.enter_context(tc.tile_pool(name="pstr", bufs=2, space="PSUM"))

        # ---- constants ----
        ident = const.tile([P, P], BF)
        make_identity(nc, ident)
        ones_c = const.tile([P, 1], BF)
        nc.vector.memset(ones_c, 1.0)
        ones_r = const.tile([1, P], BF)
        nc.vector.memset(ones_r, 1.0)
        eps_c = const.tile([P, 1], F32)
        nc.vector.memset(eps_c, EPS)
        eps_1 = const.tile([1, 1], F32)
        nc.vector.memset(eps_1, EPS)
        magic_i = const.tile([P, 1], I32)
        nc.vector.memset(magic_i, 0x5f3759df)
        one_i = const.tile([P, 1], I32)
        nc.vector.memset(one_i, 1)
        cs = const.tile([P, L], BF)               # rows 0-63 cos, 64-127 sin
        csw = const.tile([P, L], BF)              # rows 0-63 sin, 64-127 cos
        qs = const.tile([P, 1], F32)
        ks = const.tile([P, 1], F32)
        b1t = const.tile([P, NBLK1], F32)
        b2t = const.tile([P, HC], F32)

        # persistent small modulation tiles
        scale1p = modp.tile([P, HC], F32)
        shift_b = modp.tile([P, HC], BF)
        gate_t = modp.tile([P, HC], F32)
        btot = modp.tile([P, NBLK1], F32)

        atp = top.enter_context(tc.tile_pool(name="attn", bufs=1))
        glp = top.enter_context(tc.tile_pool(name="glp", bufs=1))
        gelT_d = dram.tile([NBLK1 - 9, P, L], BF)

        with ExitStack() as bc_scope:
            qkv = bc_scope.enter_context(tc.tile_pool(name="qkv", bufs=1))
            qkT = [qkv.tile([P, L], BF, tag=f"q{h}", name=f"q{h}") for h in range(H_PER)] + \
                  [qkv.tile([P, L], BF, tag=f"k{h}", name=f"k{h}") for h in range(H_PER)]
            vbT = [qkv.tile([P, NKB, P], BF, tag=f"vb{h}", name=f"vb{h}")
                   for h in range(H_PER)]

            # ============================================================
            # Phase A: modulation matvec (sharded) + AllGather
            # ============================================================
            with ExitStack() as ab:
                aa = ab.enter_context(ExitStack())
                mvp = aa.enter_context(tc.tile_pool(name="mvp", bufs=3))
                svf = modp.tile([P, HC], F32)
                nc.sync.dma_start(out=svf, in_=vec_in.rearrange("(c p) -> p c", p=P))
                sv = modp.tile([P, HC], BF)
                nc.scalar.activation(sv, svf, AF.Silu)
                msh = modp.tile([1, MODSH], F32)
                mwt_r = mwt_in.rearrange("(c p) m -> p c m", p=P)
                MJ = MODSH // 3  # 384
                for j in range(3):
                    mwt = mvp.tile([P, HC, MJ], BF, tag="mwt")
                    nc.sync.dma_start(out=mwt, in_=mwt_r[:, :, j * MJ:(j + 1) * MJ])
                    ps = pscol.tile([1, MJ], F32, tag="col")
                    for hc in range(HC):
                        nc.tensor.matmul(ps, sv[:, hc:hc + 1], mwt[:, hc],
                                         start=(hc == 0), stop=(hc == HC - 1))
                    nc.scalar.copy(msh[:, j * MJ:(j + 1) * MJ], ps)
                m_shard = dram.tile([MODSH], F32)
                nc.sync.dma_start(out=m_shard.rearrange("(a b) -> a b", a=1),
                                  in_=msh)
                # cold constants load after the modulation path is queued
                nc.sync.dma_start(out=qs, in_=qs_in[:, None])
                nc.sync.dma_start(out=ks, in_=ks_in[:, None])
                nc.sync.dma_start(out=b1t, in_=b1_in.rearrange("(b p) -> p b", p=P))
                nc.sync.dma_start(out=b2t, in_=b2_in.rearrange("(b p) -> p b", p=P))
                nc.sync.dma_start(out=cs, in_=cs_in[:, :])
                nc.sync.dma_start(out=csw, in_=csw_in[:, :])
                m_full = dram.tile([3 * HID], F32)
                if os.environ.get("KNOCOLL"):
                    nc.sync.dma_start(
                        out=m_full[0:MODSH].rearrange("(a b) -> a b", a=1), in_=msh)
                else:
                    nc.gpsimd.collective_compute(
                        "AllGather", ALU.bypass, replica_groups=[list(range(NCORES))],
                        ins=[m_shard.opt()], outs=[m_full.opt()])
                nc.gpsimd.dma_start(out=scale1p,
                                    in_=m_full[HID:2 * HID].rearrange("(c p) -> p c", p=P))
                nc.vector.tensor_scalar_add(scale1p, scale1p, 1.0)
                shift_f = modp.tile([P, HC], F32)
                nc.gpsimd.dma_start(out=shift_f,
                                    in_=m_full[0:HID].rearrange("(c p) -> p c", p=P))
                nc.vector.tensor_copy(shift_b, shift_f)
                nc.gpsimd.dma_start(out=gate_t,
                                    in_=m_full[2 * HID:3 * HID].rearrange("(c p) -> p c", p=P))
                aa.close()

                # ============================================================
                # Phase B: per quarter: LN -> x_modT(bf16) -> linear1
                # Phase C (rms+rope) interleaved into quarter 3's mlp blocks
                # ============================================================
                lnp = ab.enter_context(tc.tile_pool(name="ln", bufs=2))
                lnx = ab.enter_context(tc.tile_pool(name="lnx", bufs=1))
                lns = ab.enter_context(tc.tile_pool(name="lns", bufs=2))
                xmp = ab.enter_context(tc.tile_pool(name="xm", bufs=2))
                w1p = ab.enter_context(tc.tile_pool(name="w1s", bufs=2))
                vqp = ab.enter_context(tc.tile_pool(name="vq", bufs=1))
                # C pools (used interleaved within quarter 3)
                rmsp = ab.enter_context(tc.tile_pool(name="rms", bufs=1))
                srp = ab.enter_context(tc.tile_pool(name="srp", bufs=2))
                srp8 = ab.enter_context(tc.tile_pool(name="srp8", bufs=9))
                rbp = ab.enter_context(tc.tile_pool(name="rbp", bufs=1))
                rtp = ab.enter_context(tc.tile_pool(name="rtp", bufs=3))

                x_r = x_in.rearrange("(t p) h -> t p h", p=P)
                w1t_r = w1t_in.rearrange("(c p) m -> p c m", p=P)

                def rms_part1(i):
                    """QK-norm sum-of-squares + rsqrt rows via scalar
                    exp(-0.5*ln(ssq)); broadcast/rope deferred to part2 so
                    the PE never waits on the scalar chain."""
                    t = qkT[i]
                    sq = rmsp.tile([P, L], BF, tag="sq")
                    nc.vector.tensor_mul(sq, t, t)
                    rinvs = []
                    for j in range(NQ):
                        jsl = slice(j * LQ, (j + 1) * LQ)
                        psd = pscol.tile([1, LQ], F32, tag="col")
                        nc.tensor.matmul(psd, ones_c, sq[:, jsl],
                                         start=True, stop=True)
                        # rinv = exp(-0.5*ln(mean_sq + eps)) = rsqrt
                        srt = srp.tile([1, LQ], F32, tag="srt")
                        nc.scalar.activation(srt, psd, AF.Ln,
                                             bias=eps_1, scale=1.0 / HD)
                        rinv = srp8.tile([1, LQ], BF, tag="rinv")
                        nc.scalar.activation(rinv, srt, AF.Exp, scale=-0.5)
                        rinvs.append(rinv)
                    return rinvs

                def rms_part2(i, rinvs):
                    t = qkT[i]
                    scale_ap = qs if i < H_PER else ks
                    rb = rbp.tile([P, L], BF, tag="rb")
                    for j in range(NQ):
                        jsl = slice(j * LQ, (j + 1) * LQ)
                        pb = psum.tile([P, LQ], F32, tag="big")
                        nc.tensor.matmul(pb, ones_r, rinvs[j],
                                         start=True, stop=True)
                        nc.scalar.activation(rb[:, jsl], pb, AF.Copy,
                                             scale=scale_ap)
                    nc.vector.tensor_mul(t, t, rb)
                    # rope: rows 0-63 even pair components, 64-127 odd.
                    te, to = t[0:64, :], t[64:128, :]
                    A = rtp.tile([P, L], BF, tag="rt")   # [qe*cos ; qo*cos]
                    B = rtp.tile([P, L], BF, tag="rt")   # [qe*sin ; qo*sin]
                    Bx = rtp.tile([P, L], BF, tag="rt")  # [qo*sin ; qe*sin]
                    nc.vector.tensor_mul(A[0:64, :], te, cs[0:64, :])
                    nc.vector.tensor_mul(A[64:128, :], to, csw[64:128, :])
                    nc.vector.tensor_mul(B[0:64, :], te, csw[0:64, :])
                    nc.vector.tensor_mul(B[64:128, :], to, cs[64:128, :])
                    nc.sync.dma_start(out=Bx[0:64, :], in_=B[64:128, :])
                    nc.sync.dma_start(out=Bx[64:128, :], in_=B[0:64, :])
                    nc.vector.tensor_tensor(te, A[0:64, :], Bx[0:64, :], ALU.subtract)
                    nc.vector.tensor_tensor(to, Bx[64:128, :], A[64:128, :], ALU.add)

                def lin1_block(blk, w1t, wsub, xmT, q):
                    qsl = slice(q * LQ, (q + 1) * LQ)
                    ps = psum.tile([P, LQ], F32, tag="big")
                    for hc in range(HC):
                        nc.tensor.matmul(ps, w1t[:, hc, wsub * P:(wsub + 1) * P],
                                         xmT[:, hc, :],
                                         start=(hc == 0), stop=(hc == HC - 1))
                    if q == 0:
                        psb = pscol.tile([P, 1], F32, tag="col")
                        for hc in range(HC):
                            nc.tensor.matmul(psb, w1t[:, hc, wsub * P:(wsub + 1) * P],
                                             shift_b[:, hc:hc + 1],
                                             start=(hc == 0), stop=(hc == HC - 1))
                        nc.vector.tensor_tensor(btot[:, blk:blk + 1], psb,
                                                b1t[:, blk:blk + 1], ALU.add)
                    if blk < 6:       # q / k
                        nc.vector.tensor_scalar_add(qkT[blk][:, qsl], ps,
                                                    btot[:, blk:blk + 1])
                    elif blk < 9:     # v: evict then transpose to [l, d]
                        h = blk - 6
                        vq = vqp.tile([P, LQ], BF, tag="vq")
                        nc.vector.tensor_scalar_add(vq, ps, btot[:, blk:blk + 1])
                        ptv = pstr.tile([P, LB, P], BF, tag="tr")
                        for j in range(LB):
                            nc.tensor.transpose(ptv[:, j], vq[:, j * P:(j + 1) * P],
                                                ident)
                        nc.scalar.copy(vbT[h][:, q * LB:(q + 1) * LB], ptv)
                    else:             # mlp -> gelu -> DRAM spill (bf16)
                        g = vqp.tile([P, LQ], BF, tag="gel")
                        nc.scalar.activation(g, ps, AF.Gelu_apprx_tanh,
                                             bias=btot[:, blk:blk + 1])
                        nc.sync.dma_start(out=gelT_d[blk - 9, :, qsl], in_=g)

                NPAIR = (NBLK1 + 1) // 2  # 11 (last is a single)

                def emit_ln(q):
                    xmT = xmp.tile([P, HC, LQ], BF, tag="xmT")
                    for lb in range(LB):
                        ti = q * LB + lb
                        xt = lnp.tile([P, HID], F32, tag="xt")
                        nc.sync.dma_start(out=xt, in_=x_r[ti])
                        stats = lns.tile([P, 6, 6], F32, tag="stats")
                        for sg in range(6):
                            nc.vector.bn_stats(out=stats[:, sg, :],
                                               in_=xt[:, sg * 512:(sg + 1) * 512])
                        mv = lns.tile([P, 2], F32, tag="mv")
                        nc.vector.bn_aggr(out=mv, in_=stats)
                        # rstd = rsqrt(var+eps) on DVE: bit-trick seed +
                        # two Newton steps (keeps ScalarE free of Sqrt table
                        # loads that thrash against the Gelu table)
                        v = lns.tile([P, 1], F32, tag="v")
                        nc.vector.tensor_scalar_add(v, mv[:, 1:2], EPS)
                        yi = lns.tile([P, 1], I32, tag="yi")
                        nc.vector.tensor_scalar(yi, v.bitcast(I32), one_i,
                                                None, ALU.arith_shift_right)
                        nc.vector.tensor_tensor(yi, magic_i, yi, ALU.subtract)
                        y = yi.bitcast(F32)
                        ab_t = lns.tile([P, 1], F32, tag="ab")
                        rstd = lns.tile([P, 1], F32, tag="rstd")
                        for it in range(2):
                            nc.vector.tensor_tensor(ab_t, v, y, ALU.mult)
                            nc.vector.tensor_tensor(ab_t, ab_t, y, ALU.mult)
                            nc.vector.tensor_scalar(ab_t, ab_t, -0.5, 1.5,
                                                    ALU.mult, ALU.add)
                            dst = y if it == 0 else rstd
                            nc.vector.tensor_tensor(dst, y, ab_t, ALU.mult)
                        xn = lnx.tile([P, HID], BF, tag="xn")
                        nc.vector.tensor_scalar(xn, xt, mv[:, 0:1],
                                                rstd, ALU.subtract, ALU.mult)
                        # transpose 4 chunks into one PSUM bank, evict in one
                        # plain copy (scale1p applied per-quarter afterwards)
                        for hg in range(HC // 4):
                            pt = pstr.tile([P, 4, P], BF, tag="tr")
                            for j in range(4):
                                nc.tensor.transpose(
                                    pt[:, j], xn[:, (hg * 4 + j) * P:
                                                  (hg * 4 + j + 1) * P], ident)
                            nc.scalar.copy(
                                xmT[:, hg * 4:(hg + 1) * 4, lb * P:(lb + 1) * P],
                                pt)
                    for hcc in range(HC):
                        nc.vector.tensor_scalar_mul(xmT[:, hcc], xmT[:, hcc],
                                                    scale1p[:, hcc:hcc + 1])
                    return xmT

                def emit_blocks(q, xmT):
                    for pb in range(NPAIR):
                        wid = 2 if pb < NPAIR - 1 else 1
                        w1t = w1p.tile([P, HC, wid * P], BF, tag="w1t")
                        nc.sync.dma_start(
                            out=w1t,
                            in_=w1t_r[:, :, pb * 2 * P:(pb * 2 + wid) * P])
                        for wsub in range(wid):
                            lin1_block(pb * 2 + wsub, w1t, wsub, xmT, q)
                        # interleave rms+rope into quarter 3 after v done
                        if q == NQ - 1 and 4 <= pb < 10:
                            i = pb - 4
                            if i > 0:
                                rms_part2(i - 1, rms_st.pop(0))
                            rms_st.append(rms_part1(i))
                    if q == NQ - 1:
                        rms_part2(5, rms_st.pop(0))

                rms_st = []
                # software pipeline: LN of quarter q+1 is emitted before the
                # linear1 blocks of quarter q so its DVE/transpose work hides
                # under the previous quarter's matmul stream
                xm_next = emit_ln(0)
                for q in range(NQ):
                    xm_cur = xm_next
                    if q + 1 < NQ:
                        xm_next = emit_ln(q + 1)
                    if q == 0:
                        # keep the PE's HAM activity window busy while the
                        # modulation AllGather completes, so linear1 starts
                        # at full clock instead of 4/8 throttle
                        for _ in range(250):
                            dm = pstr.tile([P, P], F32, tag="tr")
                            nc.tensor.matmul(dm, ident, ident,
                                             start=True, stop=True)
                    emit_blocks(q, xm_cur)

            # ============================================================
            # Phase E: attention per head (scoresT -> exp -> denom -> outT)
            # ============================================================
            aoT = [atp.tile([P, L], BF, tag=f"ao{h}", name=f"ao{h}")
                   for h in range(H_PER)]
            # prefetch gelu spill back into SBUF during attention
            gelT = [glp.tile([P, L], BF, tag=f"gl{i}", name=f"gl{i}")
                    for i in range(NBLK1 - 9)]
            for i in range(NBLK1 - 9):
                nc.sync.dma_start(out=gelT[i], in_=gelT_d[i])
            with ExitStack() as ec:
                ptp = ec.enter_context(tc.tile_pool(name="ptp", bufs=36))
                sdp = ec.enter_context(tc.tile_pool(name="sdp", bufs=2))

                def attn_scores(h, qc):
                    qT, kT = qkT[h], qkT[H_PER + h]
                    qsl = slice(qc * LQ, (qc + 1) * LQ)
                    pts = []
                    for kb in range(NKB):
                        ps = psum.tile([P, LQ], F32, tag="big")
                        nc.tensor.matmul(ps, kT[:, kb * P:(kb + 1) * P],
                                         qT[:, qsl], start=True, stop=True)
                        ptile = ptp.tile([P, LQ], BF, tag="pt", name="pt")
                        nc.scalar.activation(ptile, ps, AF.Exp)
                        pts.append(ptile)
                    return pts

                def attn_denom(h, qc, pts):
                    psd = pscol.tile([1, LQ], F32, tag="col")
                    for kb in range(NKB):
                        nc.tensor.matmul(psd, ones_c, pts[kb],
                                         start=(kb == 0), stop=(kb == NKB - 1))
                    lnd = sdp.tile([1, LQ], F32, tag="lnd")
                    nc.scalar.activation(lnd, psd, AF.Ln)
                    rd = sdp.tile([1, LQ], BF, tag="rd")
                    nc.scalar.activation(rd, lnd, AF.Exp, scale=-1.0)
                    return rd

                def attn_av(h, qc, pts, rd):
                    qsl = slice(qc * LQ, (qc + 1) * LQ)
                    pbd = psum.tile([P, LQ], F32, tag="big")
                    nc.tensor.matmul(pbd, ones_r, rd, start=True, stop=True)
                    rbd = sdp.tile([P, LQ], F32, tag="rbd")
                    nc.vector.tensor_copy(rbd, pbd)
                    pso = psum.tile([P, LQ], F32, tag="big")
                    for kb in range(NKB):
                        nc.tensor.matmul(pso, vbT[h][:, kb], pts[kb],
                                         start=(kb == 0), stop=(kb == NKB - 1))
                    nc.vector.tensor_mul(aoT[h][:, qsl], pso, rbd)

                chunks = [(h, qc) for h in range(H_PER) for qc in range(NQ)]
                st = []  # [(chunk, pts, rd?)] pipeline stages
                for ch in chunks:
                    if len(st) >= 2:
                        c0, p0, r0 = st.pop(0)
                        attn_av(*c0, p0, r0)
                    pts = attn_scores(*ch)
                    if st:
                        st[-1][2] = attn_denom(*st[-1][0], st[-1][1])
                    st.append([ch, pts, None])
                c0, p0, r0 = st.pop(0)   # denom already emitted in-loop
                attn_av(*c0, p0, r0)
                c1, p1, _ = st.pop(0)
                r1 = attn_denom(*c1, p1)
                attn_av(*c1, p1, r1)

        # ============================================================
        # Phase F: linear2 (row-parallel partial) with gate; +b2 on core 0
        # ============================================================
        with ExitStack() as fc:
            w2p = fc.enter_context(tc.tile_pool(name="w2p", bufs=2))
            otp = fc.enter_context(tc.tile_pool(name="otp", bufs=2))
            catT = aoT + gelT  # 15 chunks of [128, L]
            w2t_r = w2t_in.rearrange("(c p) m -> p c m", p=P)
            for pb in range(HC // 2):
                w2t = w2p.tile([P, NCAT, 2 * P], BF, tag="w2t")
                nc.sync.dma_start(out=w2t,
                                  in_=w2t_r[:, :, pb * 2 * P:(pb + 1) * 2 * P])
                for wsub in range(2):
                    blk = pb * 2 + wsub
                    ot = otp.tile([P, L], F32, tag="ot")
                    for lc in range(NQ):
                        lsl = slice(lc * LQ, (lc + 1) * LQ)
                        ps = psum.tile([P, LQ], F32, tag="big")
                        for hc in range(NCAT):
                            nc.tensor.matmul(ps, w2t[:, hc, wsub * P:(wsub + 1) * P],
                                             catT[hc][:, lsl],
                                             start=(hc == 0), stop=(hc == NCAT - 1))
                        nc.vector.tensor_scalar(ot[:, lsl], ps, b2t[:, blk:blk + 1],
                                                gate_t[:, blk:blk + 1],
                                                ALU.add, ALU.mult)
                    nc.sync.dma_start(out=out_t[blk], in_=ot)


def _host_prep(inputs):
    bf = ml_dtypes.bfloat16
    perm = np.concatenate([np.arange(0, HD, 2), np.arange(1, HD, 2)])
    w1 = inputs["w1"].astype(np.float32)
    w1_eff = w1.copy()
    for i, nm in enumerate(("q", "k", "v")):
        up = inputs[f"lora_{nm}_up"].astype(np.float32)
        dn = inputs[f"lora_{nm}_down"].astype(np.float32)
        w1_eff[i * HID:(i + 1) * HID] += up @ dn
    b1 = inputs["b1"].astype(np.float32)
    w2 = inputs["w2"].astype(np.float32)
    w2_eff = w2 + inputs["proj_up"].astype(np.float32) @ \
        inputs["proj_down"].astype(np.float32)
    mod_w = inputs["mod_w"].astype(np.float32)
    mod_b = inputs["mod_b"].astype(np.float32)
    if np.abs(mod_b).max() != 0.0:
        raise NotImplementedError("nonzero mod_b not supported")

    wq = w1_eff[0:HID].reshape(HEADS, HD, HID)[:, perm, :]
    wk = w1_eff[HID:2 * HID].reshape(HEADS, HD, HID)[:, perm, :]
    wv = w1_eff[2 * HID:3 * HID].reshape(HEADS, HD, HID)
    wm = w1_eff[3 * HID:].reshape(NCORES, DMLP, HID)
    bq = b1[0:HID].reshape(HEADS, HD)[:, perm]
    bk = b1[HID:2 * HID].reshape(HEADS, HD)[:, perm]
    bv = b1[2 * HID:3 * HID].reshape(HEADS, HD)
    bm = b1[3 * HID:].reshape(NCORES, DMLP)

    pe = inputs["pe"].astype(np.float32)
    cos = pe[0, 0, :, :, 0, 0]   # (L, 64)
    sin = pe[0, 0, :, :, 1, 0]   # (L, 64)
    cs = np.ascontiguousarray(
        np.concatenate([cos.T, sin.T], axis=0)).astype(bf)  # (128, L)
    csw = np.ascontiguousarray(
        np.concatenate([sin.T, cos.T], axis=0)).astype(bf)  # (128, L)

    qsc = inputs["q_scale"].astype(np.float32)[perm] / math.sqrt(HD)
    ksc = inputs["k_scale"].astype(np.float32)[perm]
    x2d = np.ascontiguousarray(inputs["x"].astype(np.float32).reshape(L, HID))
    vecv = np.ascontiguousarray(inputs["vec"].astype(np.float32).reshape(HID))
    b2 = inputs["b2"].astype(np.float32)

    in_maps = []
    for c in range(NCORES):
        hs = slice(H_PER * c, H_PER * (c + 1))
        w1s = np.concatenate([
            wq[hs].reshape(DQK, HID), wk[hs].reshape(DQK, HID),
            wv[hs].reshape(DQK, HID), wm[c]], axis=0)
        b1s = np.concatenate([
            bq[hs].reshape(DQK), bk[hs].reshape(DQK), bv[hs].reshape(DQK), bm[c]])
        w2s = np.concatenate([
            w2_eff[:, DQK * c:DQK * (c + 1)],
            w2_eff[:, HID + DMLP * c:HID + DMLP * (c + 1)]], axis=1)
        assert w2s.shape == (HID, CATD), w2s.shape
        in_maps.append({
            "x_in": x2d,
            "vec_in": vecv,
            "cs_in": cs,
            "csw_in": csw,
            "w1t_in": np.ascontiguousarray(w1s.T).astype(bf),
            "b1_in": np.ascontiguousarray(b1s),
            "w2t_in": np.ascontiguousarray(w2s.T).astype(bf),
            "b2_in": b2 if c == 0 else np.zeros_like(b2),
            "mwt_in": np.ascontiguousarray(
                mod_w[MODSH * c:MODSH * (c + 1)].T).astype(bf),
            "qs_in": qsc,
            "ks_in": ksc,
        })
    return in_maps


def kernel(**inputs):
    if "nc" not in _CACHED:
        _CACHED["nc"] = _build_nc()
    nc = _CACHED["nc"]
    in_maps = _host_prep(inputs)
    res = run_bass_kernel_spmd(nc, in_maps, core_ids=list(range(NCORES)))
    acc = np.zeros((HID, L), dtype=np.float64)
    for c in range(NCORES):
        acc += res.results[c]["out_part"].reshape(HID, L)
    out = inputs["x"].astype(np.float32).reshape(L, HID) + acc.T.astype(np.float32)
    return out.reshape(1, L, HID).astype(np.float32)
